# revision 1
# baseline (speedup 1.0000x reference)
"""Trainium2 Bass kernel for a dense (length-1 sequence) Mamba block.

The reference computation reduces algebraically to:
    z   = x @ in_w                                  # (B, d_inner)
    g   = silu(z * c + b_eff)                       # per-channel scale/bias
    out = g @ out_w + out_b                         # (B, d_model)
with
    c     = conv_w[:, -1] + softplus(dt) * sum(B*C, -1) + Dp
    b_eff = (in_b * c) + conv_b
(c, b_eff are tiny per-channel vectors, computed on host.)

Strategy: data-parallel over 8 NeuronCores (batch 32768 -> 8 x 4096).
Per core, batch is processed in tiles of BT rows:
  phase T : PE-transpose x tiles into xT [d_model, BT] layout
  phase M1: z^T[di, b] accumulated over d_model via float32r (FP22) matmuls
            with in_w tiles as the stationary operand; Silu fused on ScalarE
            with per-partition scale/bias -> g [di, b]
  phase M2: out[b, dm] accumulated over d_inner with g slices as the
            stationary operand and out_w tiles moving (natural output
            layout; no output transpose needed); out_b added on drain.
"""

import numpy as np

import concourse.bass as bass
import concourse.tile as tile
from concourse import bacc, mybir
from concourse.bass_utils import run_bass_kernel_spmd

P = 128
B_FULL = 32768
DM = 2048
DI = 4096
N_CORES = 8
BS = B_FULL // N_CORES  # rows per core

F32 = mybir.dt.float32
F32R = mybir.dt.float32r
BF16 = mybir.dt.bfloat16
SILU = mybir.ActivationFunctionType.Silu


# float32r (FP22) tensors: same fp32 bytes in DRAM/numpy, but instructions
# producing them round to FP22 so the full-speed reduced-precision matmul
# path can consume them (walrus verifier requirement).


def build_nc(cfg):
    """Build the per-core Bass module. cfg: dict(BT=..., g_bf16=..., ow_bf16=...)"""
    BT = cfg["BT"]
    g_dt = BF16 if cfg["g_bf16"] else F32R
    ow_dt = BF16 if cfg["ow_bf16"] else F32R

    NBT = BS // BT          # batch tiles per core
    NB_SUB = BT // P        # 128-row subtiles per batch tile
    KT = DM // P            # k-tiles for matmul 1
    NDI = DI // P           # d_inner chunks of 128
    NDM = DM // 512         # d_model chunks of 512
    H = BT // 512           # moving-dim halves for matmul 1
    GRP = 4                 # psum banks used by M2 accumulation
    NGRP = NB_SUB // GRP

    nc = bacc.Bacc("TRN2", target_bir_lowering=False, debug=False,
                   num_devices=N_CORES)

    x_d = nc.dram_tensor("x", [BS, DM], F32R, kind="ExternalInput").ap()
    iw_d = nc.dram_tensor("iw", [DM, DI], F32R, kind="ExternalInput").ap()
    ow_d = nc.dram_tensor("ow", [DI, DM], ow_dt, kind="ExternalInput").ap()
    c_d = nc.dram_tensor("cpb", [P, NDI], F32, kind="ExternalInput").ap()
    b_d = nc.dram_tensor("bpb", [P, NDI], F32, kind="ExternalInput").ap()
    ob_d = nc.dram_tensor("ob", [P, DM], F32, kind="ExternalInput").ap()
    id_d = nc.dram_tensor("ident", [P, P], F32R, kind="ExternalInput").ap()
    out_d = nc.dram_tensor("out", [BS, DM], F32, kind="ExternalOutput").ap()

    DIG = 4                 # d_inner chunks per out_w DMA batch
    with tile.TileContext(nc) as tc:
        with (
            tc.tile_pool(name="const", bufs=1) as const,
            tc.tile_pool(name="xnat", bufs=2) as xnat,
            tc.tile_pool(name="xT", bufs=1) as xTp,
            tc.tile_pool(name="g", bufs=1) as gp,
            tc.tile_pool(name="iw", bufs=3) as iwp,
            tc.tile_pool(name="ow", bufs=3) as owp,
            tc.tile_pool(name="osb", bufs=2) as osbp,
            tc.tile_pool(name="psZ", bufs=3, space="PSUM") as psZ,
            tc.tile_pool(name="psO", bufs=5, space="PSUM") as psO,
        ):
            ident = const.tile([P, P], F32R)
            nc.sync.dma_start(ident[:], id_d)
            c_sb = const.tile([P, NDI], F32)
            nc.sync.dma_start(c_sb[:], c_d)
            b_sb = const.tile([P, NDI], F32)
            nc.sync.dma_start(b_sb[:], b_d)
            ob_sb = const.tile([P, DM], F32)
            nc.sync.dma_start(ob_sb[:], ob_d)

            xT = xTp.tile([P, KT, BT], F32R)
            g = gp.tile([P, NDI, BT], g_dt)

            def emit_T(t, bs):
                """Transpose one 128-row block of x[t] into xT."""
                xn = xnat.tile([P, DM], F32R, tag="xn", name="xn")
                nc.gpsimd.dma_start(xn[:], x_d[t * BT + bs * P:
                                               t * BT + (bs + 1) * P, :])
                for kq in range(KT // 4):
                    pst = psO.tile([P, 4, P], F32R, tag="ps_o", name="pst")
                    for q in range(4):
                        kt = kq * 4 + q
                        nc.tensor.transpose(
                            pst[:, q, :], xn[:, kt * P:(kt + 1) * P],
                            ident[:])
                    nc.vector.tensor_copy(
                        out=xT[:, kq * 4:(kq + 1) * 4, bs * P:(bs + 1) * P],
                        in_=pst[:])

            for t in range(NBT):
                if t == 0:
                    # prologue: transpose the first batch tile up front
                    for bs in range(NB_SUB):
                        emit_T(0, bs)

                # ---- phase M1: z^T = in_w^T @ x^T ; g = silu(z*c + b) ----
                for di in range(NDI):
                    iw_t = iwp.tile([P, KT, P], F32R)
                    nc.scalar.dma_start(
                        iw_t[:],
                        iw_d[:, di * P:(di + 1) * P].rearrange(
                            "(kt p) m -> p kt m", p=P))
                    zps = [psZ.tile([P, 512], F32, tag="zp", name=f"zp_{h}")
                           for h in range(H)]
                    for kt in range(KT):
                        for h in range(H):
                            nc.tensor.matmul(
                                zps[h][:],
                                iw_t[:, kt, :],
                                xT[:, kt, h * 512:(h + 1) * 512],
                                start=(kt == 0), stop=(kt == KT - 1))
                    for h in range(H):
                        nc.scalar.activation(
                            g[:, di, h * 512:(h + 1) * 512], zps[h][:], SILU,
                            bias=b_sb[:, di:di + 1], scale=c_sb[:, di:di + 1])

                # ---- phase M2: out = g^T @ out_w + out_b ----
                # T-units for batch tile t+1 interleaved after each psum
                # group: transposes fill PE slack, x DMAs spread over the
                # whole M2 window.
                ui = 0
                for dmc in range(NDM):
                    for grp in range(NGRP):
                        ops = [psO.tile([P, 512], F32, tag="ps_o",
                                        name=f"ops_{j}")
                               for j in range(GRP)]
                        for dg in range(NDI // DIG):
                            ow_t = owp.tile([P, DIG, 512], ow_dt)
                            nc.sync.dma_start(
                                ow_t[:],
                                ow_d[dg * DIG * P:(dg + 1) * DIG * P,
                                     dmc * 512:(dmc + 1) * 512].rearrange(
                                         "(s p) n -> p s n", p=P))
                            for s in range(DIG):
                                di = dg * DIG + s
                                for j in range(GRP):
                                    bs = grp * GRP + j
                                    nc.tensor.matmul(
                                        ops[j][:],
                                        g[:, di, bs * P:(bs + 1) * P],
                                        ow_t[:, s, :],
                                        start=(di == 0),
                                        stop=(di == NDI - 1))
                        osb = osbp.tile([P, GRP, 512], F32)
                        for j in range(GRP):
                            nc.vector.tensor_tensor(
                                osb[:, j, :], ops[j][:],
                                ob_sb[:, dmc * 512:(dmc + 1) * 512],
                                mybir.AluOpType.add)
                        r0 = t * BT + grp * GRP * P
                        nc.scalar.dma_start(
                            out_d[r0:r0 + GRP * P,
                                  dmc * 512:(dmc + 1) * 512].rearrange(
                                      "(s p) n -> p s n", p=P),
                            osb[:])
                        if t + 1 < NBT and ui < NB_SUB:
                            emit_T(t + 1, ui)
                            ui += 1
    nc.compile()
    return nc


_NC_CACHE = {}


def _get_nc(key):
    if key not in _NC_CACHE:
        cfg = dict(BT=key[0], g_bf16=key[1], ow_bf16=key[2])
        _NC_CACHE[key] = build_nc(cfg)
    return _NC_CACHE[key]


# default config: fp32r matmul-1, bf16 g + out_w for matmul-2, BT=1024
CONFIG = (1024, True, True)


def _softplus(v):
    return np.logaddexp(0.0, v)


def kernel(x, in_w, in_b, conv_w, conv_b, A_log, B, C, Dp, dt, out_w, out_b):
    x = np.asarray(x, dtype=np.float32)
    in_w = np.ascontiguousarray(np.asarray(in_w, dtype=np.float32))
    out_w = np.asarray(out_w, dtype=np.float32)

    # host precompute of the per-channel SSM/conv collapse
    c = (np.asarray(conv_w, np.float32)[:, -1]
         + _softplus(np.asarray(dt, np.float32))
         * np.sum(np.asarray(B, np.float32) * np.asarray(C, np.float32), -1)
         + np.asarray(Dp, np.float32))
    b_eff = np.asarray(in_b, np.float32) * c + np.asarray(conv_b, np.float32)

    # [128, DI//128] partition-major layouts for per-partition scale/bias
    c_pb = np.ascontiguousarray(c.reshape(DI // P, P).T)
    b_pb = np.ascontiguousarray(b_eff.reshape(DI // P, P).T)
    ob_rep = np.ascontiguousarray(
        np.broadcast_to(np.asarray(out_b, np.float32), (P, DM)))

    key = CONFIG
    nc = _get_nc(key)
    if key[2]:
        import ml_dtypes
        ow_arr = out_w.astype(ml_dtypes.bfloat16)
    else:
        ow_arr = np.ascontiguousarray(out_w)

    in_maps = []
    for i in range(N_CORES):
        in_maps.append({
            "x": np.ascontiguousarray(x[i * BS:(i + 1) * BS]),
            "iw": in_w,
            "ow": ow_arr,
            "cpb": c_pb,
            "bpb": b_pb,
            "ob": ob_rep,
            "ident": np.eye(P, dtype=np.float32),
        })
    out = np.empty((B_FULL, DM), dtype=np.float32)
    try:
        res = run_bass_kernel_spmd(nc, in_maps, core_ids=list(range(N_CORES)))
        for i in range(N_CORES):
            out[i * BS:(i + 1) * BS] = res.results[i]["out"]
    except Exception:
        # The accelerator occasionally hits a transient unrecoverable fault
        # that poisons this process's PJRT client; a fresh process recovers.
        # Retry the device execution in a subprocess.
        _run_in_subprocess(in_maps, out)
    return out


def _run_in_subprocess(in_maps, out):
    import pickle
    import subprocess
    import sys
    import tempfile

    with tempfile.TemporaryDirectory() as td:
        in_path = f"{td}/in.pkl"
        out_path = f"{td}/out.npy"
        with open(in_path, "wb") as f:
            pickle.dump({"config": CONFIG, "in_maps": in_maps}, f,
                        protocol=pickle.HIGHEST_PROTOCOL)
        for attempt in range(3):
            r = subprocess.run(
                [sys.executable, __file__, "--worker", in_path, out_path],
                capture_output=True)
            if r.returncode == 0:
                break
            if attempt == 2:
                raise RuntimeError(
                    f"device worker failed 3x: {r.stderr[-2000:]!r}")
        out[:] = np.load(out_path)


def _worker_main(in_path, out_path):
    import pickle
    with open(in_path, "rb") as f:
        job = pickle.load(f)
    nc = _get_nc(tuple(job["config"]))
    res = run_bass_kernel_spmd(nc, job["in_maps"],
                               core_ids=list(range(N_CORES)))
    out = np.empty((B_FULL, DM), dtype=np.float32)
    for i in range(N_CORES):
        out[i * BS:(i + 1) * BS] = res.results[i]["out"]
    np.save(out_path, out)


if __name__ == "__main__":
    import sys as _sys
    if len(_sys.argv) == 4 and _sys.argv[1] == "--worker":
        _worker_main(_sys.argv[2], _sys.argv[3])



# revision 2
# speedup vs baseline: 1.0741x; 1.0741x over previous
"""Trainium2 Bass kernel for a dense (length-1 sequence) Mamba block.

The reference computation reduces algebraically to:
    z   = x @ in_w                                  # (B, d_inner)
    g   = silu(z * c + b_eff)                       # per-channel scale/bias
    out = g @ out_w + out_b                         # (B, d_model)
with
    c     = conv_w[:, -1] + softplus(dt) * sum(B*C, -1) + Dp
    b_eff = (in_b * c) + conv_b
(c, b_eff are tiny per-channel vectors, computed on host.)

Strategy: data-parallel over 8 NeuronCores (batch 32768 -> 8 x 4096).
All matmul operands are bf16 (rel err ~3e-3, tolerance 2e-2). The x
operand is transposed and tiled on the HOST into a [t][p][kt][b]
layout, so the device PE array runs nothing but the 8192 essential
matmuls per core -- no PE transposes, no transpose psum traffic.
in_w / out_w are host-shuffled so every weight DMA is a contiguous
per-partition >=4KB burst.

Per core, per batch tile of BT=1024 rows:
  M1: z^T[di,b] += in_w^T @ x^T over 16 k-tiles (psZ, 2 banks/di,
      3-deep pool); Silu fused on ScalarE with per-partition
      scale/bias -> g[di,b] bf16.
  M2: out[b,dm] += g^T @ out_w over 32 di chunks, 4 psum banks per
      bs-group (2 groups of 4x128 rows), out_b added on the DVE
      drain; drains pipeline with the accumulation tails.
"""

import numpy as np

import concourse.bass as bass
import concourse.tile as tile
from concourse import bacc, mybir
from concourse.bass_utils import run_bass_kernel_spmd

P = 128
B_FULL = 32768
DM = 2048
DI = 4096
N_CORES = 8
BS = B_FULL // N_CORES  # rows per core

BT = 1024               # batch tile rows
NBT = BS // BT          # 4 batch tiles per core
NB_SUB = BT // P        # 8 x 128-row subtiles per batch tile
KT = DM // P            # 16 k-tiles for matmul 1
NDI = DI // P           # 32 d_inner chunks of 128
NDM = DM // 512         # 4 d_model chunks of 512
H = BT // 512           # 2 moving-dim halves for matmul 1
GRP = 4                 # psum banks per M2 bs-group
NGRP = NB_SUB // GRP    # 2 bs-groups
DIG = 4                 # d_inner chunks per out_w DMA
NDG = NDI // DIG        # 8 out_w DMA chunks per dm column block

F32 = mybir.dt.float32
BF16 = mybir.dt.bfloat16
SILU = mybir.ActivationFunctionType.Silu


def build_nc():
    nc = bacc.Bacc("TRN2", target_bir_lowering=False, debug=False,
                   num_devices=N_CORES)

    # host-shuffled layouts (see prepare_in_maps):
    #  xt : row t*128+p holds [kt][b]  (b within tile t)     bf16
    #  iw : row di*128+p holds [kt][m]                        bf16
    #  ow : row (dmc*NDG+dg)*128+p holds [s][m]               bf16
    xt_d = nc.dram_tensor("xt", [NBT * P, KT * BT], BF16,
                          kind="ExternalInput").ap()
    iw_d = nc.dram_tensor("iw", [NDI * P, KT * P], BF16,
                          kind="ExternalInput").ap()
    ow_d = nc.dram_tensor("ow", [NDM * NDG * P, DIG * 512], BF16,
                          kind="ExternalInput").ap()
    c_d = nc.dram_tensor("cpb", [P, NDI], F32, kind="ExternalInput").ap()
    b_d = nc.dram_tensor("bpb", [P, NDI], F32, kind="ExternalInput").ap()
    ob_d = nc.dram_tensor("ob", [P, DM], F32, kind="ExternalInput").ap()
    out_d = nc.dram_tensor("out", [BS, DM], F32, kind="ExternalOutput").ap()

    with tile.TileContext(nc) as tc:
        with (
            tc.tile_pool(name="const", bufs=1) as const,
            tc.tile_pool(name="xT", bufs=2) as xtp,
            tc.tile_pool(name="g", bufs=1) as gp,
            tc.tile_pool(name="iw", bufs=3) as iwp,
            tc.tile_pool(name="ow", bufs=3) as owp,
            tc.tile_pool(name="osb", bufs=2) as osbp,
            tc.tile_pool(name="psZ", bufs=3, space="PSUM") as psZ,
            tc.tile_pool(name="psO", bufs=5, space="PSUM") as psO,
        ):
            c_sb = const.tile([P, NDI], F32)
            nc.sync.dma_start(c_sb[:], c_d)
            b_sb = const.tile([P, NDI], F32)
            nc.sync.dma_start(b_sb[:], b_d)
            ob_sb = const.tile([P, DM], F32)
            nc.sync.dma_start(ob_sb[:], ob_d)

            g = gp.tile([P, NDI, BT], BF16)

            xTs = {}

            def load_xT(t):
                xTs[t] = xtp.tile([P, KT, BT], BF16, tag="xt", name="xt")
                nc.sync.dma_start(
                    xTs[t][:],
                    xt_d[t * P:(t + 1) * P, :].rearrange(
                        "p (kt b) -> p kt b", kt=KT))

            load_xT(0)
            for t in range(NBT):
                xT = xTs.pop(t)

                # ---- M1: z^T = in_w^T @ x^T ; g = silu(z*c + b) ----
                for di in range(NDI):
                    iw_t = iwp.tile([P, KT, P], BF16, tag="iw", name="iw")
                    nc.sync.dma_start(
                        iw_t[:],
                        iw_d[di * P:(di + 1) * P, :].rearrange(
                            "p (kt m) -> p kt m", kt=KT))
                    zps = [psZ.tile([P, 512], F32, tag="zp", name=f"zp{h}")
                           for h in range(H)]
                    for kt in range(KT):
                        for h in range(H):
                            nc.tensor.matmul(
                                zps[h][:],
                                iw_t[:, kt, :],
                                xT[:, kt, h * 512:(h + 1) * 512],
                                start=(kt == 0), stop=(kt == KT - 1))
                    for h in range(H):
                        nc.scalar.activation(
                            g[:, di, h * 512:(h + 1) * 512], zps[h][:], SILU,
                            bias=b_sb[:, di:di + 1], scale=c_sb[:, di:di + 1])

                if t + 1 < NBT:
                    load_xT(t + 1)

                # ---- M2: out = g^T @ out_w + out_b ----
                for dmc in range(NDM):
                    for grp in range(NGRP):
                        ops = [psO.tile([P, 512], F32, tag="ps_o",
                                        name=f"ops{j}")
                               for j in range(GRP)]
                        for dg in range(NDG):
                            ow_t = owp.tile([P, DIG, 512], BF16, tag="ow",
                                            name="ow")
                            r = (dmc * NDG + dg) * P
                            nc.scalar.dma_start(
                                ow_t[:],
                                ow_d[r:r + P, :].rearrange(
                                    "p (s m) -> p s m", s=DIG))
                            for s in range(DIG):
                                di = dg * DIG + s
                                for j in range(GRP):
                                    bs = grp * GRP + j
                                    nc.tensor.matmul(
                                        ops[j][:],
                                        g[:, di, bs * P:(bs + 1) * P],
                                        ow_t[:, s, :],
                                        start=(di == 0),
                                        stop=(di == NDI - 1))
                        osb = osbp.tile([P, GRP, 512], F32, tag="osb",
                                        name="osb")
                        for j in range(GRP):
                            nc.vector.tensor_tensor(
                                osb[:, j, :], ops[j][:],
                                ob_sb[:, dmc * 512:(dmc + 1) * 512],
                                mybir.AluOpType.add)
                        r0 = t * BT + grp * GRP * P
                        nc.gpsimd.dma_start(
                            out_d[r0:r0 + GRP * P,
                                  dmc * 512:(dmc + 1) * 512].rearrange(
                                      "(s p) n -> p s n", p=P),
                            osb[:])
    nc.compile()
    return nc


_NC_CACHE = {}


def _get_nc():
    if "nc" not in _NC_CACHE:
        _NC_CACHE["nc"] = build_nc()
    return _NC_CACHE["nc"]


def _softplus(v):
    return np.logaddexp(0.0, v)


def prepare_in_maps(inputs):
    """Host-side prep: per-channel collapse, bf16 casts, layout shuffles."""
    import ml_dtypes
    bf16 = ml_dtypes.bfloat16

    x = np.asarray(inputs["x"], np.float32)
    in_w = np.asarray(inputs["in_w"], np.float32)
    out_w = np.asarray(inputs["out_w"], np.float32)

    c = (np.asarray(inputs["conv_w"], np.float32)[:, -1]
         + _softplus(np.asarray(inputs["dt"], np.float32))
         * np.sum(np.asarray(inputs["B"], np.float32)
                  * np.asarray(inputs["C"], np.float32), -1)
         + np.asarray(inputs["Dp"], np.float32))
    b_eff = (np.asarray(inputs["in_b"], np.float32) * c
             + np.asarray(inputs["conv_b"], np.float32))

    c_pb = np.ascontiguousarray(c.reshape(NDI, P).T)
    b_pb = np.ascontiguousarray(b_eff.reshape(NDI, P).T)
    ob_rep = np.ascontiguousarray(np.broadcast_to(
        np.asarray(inputs["out_b"], np.float32), (P, DM)))

    # iw[kt*128+p, di*128+m] -> row di*128+p : [kt][m]
    iw_shuf = np.ascontiguousarray(
        in_w.astype(bf16).reshape(KT, P, NDI, P).transpose(2, 1, 0, 3)
        .reshape(NDI * P, KT * P))
    # ow[dg*512+s*128+p, dmc*512+m] -> row (dmc*NDG+dg)*128+p : [s][m]
    ow_shuf = np.ascontiguousarray(
        out_w.astype(bf16).reshape(NDG, DIG, P, NDM, 512)
        .transpose(3, 0, 2, 1, 4).reshape(NDM * NDG * P, DIG * 512))

    in_maps = []
    for i in range(N_CORES):
        xc = x[i * BS:(i + 1) * BS].astype(bf16)
        # xc[t*BT+b, kt*128+p] -> row t*128+p : [kt][b]
        xt_shuf = np.ascontiguousarray(
            xc.reshape(NBT, BT, KT, P).transpose(0, 3, 2, 1)
            .reshape(NBT * P, KT * BT))
        in_maps.append({
            "xt": xt_shuf,
            "iw": iw_shuf,
            "ow": ow_shuf,
            "cpb": c_pb,
            "bpb": b_pb,
            "ob": ob_rep,
        })
    return in_maps


def kernel(x, in_w, in_b, conv_w, conv_b, A_log, B, C, Dp, dt, out_w, out_b):
    in_maps = prepare_in_maps(dict(
        x=x, in_w=in_w, in_b=in_b, conv_w=conv_w, conv_b=conv_b,
        A_log=A_log, B=B, C=C, Dp=Dp, dt=dt, out_w=out_w, out_b=out_b))
    out = np.empty((B_FULL, DM), dtype=np.float32)
    try:
        nc = _get_nc()
        res = run_bass_kernel_spmd(nc, in_maps, core_ids=list(range(N_CORES)))
        for i in range(N_CORES):
            out[i * BS:(i + 1) * BS] = res.results[i]["out"]
    except Exception:
        # The accelerator occasionally hits a transient unrecoverable fault
        # that poisons this process's PJRT client; a fresh process recovers.
        # Retry the device execution in a subprocess.
        _run_in_subprocess(in_maps, out)
    return out


def _run_in_subprocess(in_maps, out):
    import pickle
    import subprocess
    import sys
    import tempfile

    with tempfile.TemporaryDirectory() as td:
        in_path = f"{td}/in.pkl"
        out_path = f"{td}/out.npy"
        with open(in_path, "wb") as f:
            pickle.dump({"in_maps": in_maps}, f,
                        protocol=pickle.HIGHEST_PROTOCOL)
        for attempt in range(3):
            r = subprocess.run(
                [sys.executable, __file__, "--worker", in_path, out_path],
                capture_output=True)
            if r.returncode == 0:
                break
            if attempt == 2:
                raise RuntimeError(
                    f"device worker failed 3x: {r.stderr[-2000:]!r}")
        out[:] = np.load(out_path)


def _worker_main(in_path, out_path):
    import pickle
    with open(in_path, "rb") as f:
        job = pickle.load(f)
    nc = _get_nc()
    res = run_bass_kernel_spmd(nc, job["in_maps"],
                               core_ids=list(range(N_CORES)))
    out = np.empty((B_FULL, DM), dtype=np.float32)
    for i in range(N_CORES):
        out[i * BS:(i + 1) * BS] = res.results[i]["out"]
    np.save(out_path, out)


if __name__ == "__main__":
    import sys as _sys
    if len(_sys.argv) == 4 and _sys.argv[1] == "--worker":
        _worker_main(_sys.argv[2], _sys.argv[3])


# revision 4
# speedup vs baseline: 1.1004x; 1.0246x over previous
"""Trainium2 Bass kernel for a dense (length-1 sequence) Mamba block.

The reference computation reduces algebraically to:
    z   = x @ in_w                                  # (B, d_inner)
    g   = silu(z * c + b_eff)                       # per-channel scale/bias
    out = g @ out_w + out_b                         # (B, d_model)
with
    c     = conv_w[:, -1] + softplus(dt) * sum(B*C, -1) + Dp
    b_eff = (in_b * c) + conv_b
(c, b_eff are tiny per-channel vectors, computed on host.)

Strategy: data-parallel over 8 NeuronCores (batch 32768 -> 8 x 4096).
All matmul operands are bf16 (rel err ~3e-3, tolerance 2e-2). The x
operand is transposed and tiled on the HOST into a [t][p][kt][b]
layout, so the device PE array runs nothing but the 8192 essential
matmuls per core -- no PE transposes, no transpose psum traffic.
in_w / out_w are host-shuffled so every weight DMA is a contiguous
per-partition >=4KB burst.

Per core, per batch tile of BT=1024 rows:
  M1: z^T[di,b] += in_w^T @ x^T over 16 k-tiles (psZ, 2 banks/di,
      3-deep pool); Silu fused on ScalarE with per-partition
      scale/bias -> g[di,b] bf16.
  M2: out[b,dm] += g^T @ out_w over 32 di chunks, 4 psum banks per
      bs-group (2 groups of 4x128 rows), out_b added on the DVE
      drain; drains pipeline with the accumulation tails.
"""

import numpy as np

import concourse.bass as bass
import concourse.tile as tile
from concourse import bacc, mybir
from concourse.bass_utils import run_bass_kernel_spmd

P = 128
B_FULL = 32768
DM = 2048
DI = 4096
N_CORES = 8
BS = B_FULL // N_CORES  # rows per core

BT = 1024               # batch tile rows
NBT = BS // BT          # 4 batch tiles per core
NB_SUB = BT // P        # 8 x 128-row subtiles per batch tile
KT = DM // P            # 16 k-tiles for matmul 1
NDI = DI // P           # 32 d_inner chunks of 128
NDM = DM // 512         # 4 d_model chunks of 512
H = BT // 512           # 2 moving-dim halves for matmul 1
GRP = 4                 # psum banks per M2 bs-group
NGRP = NB_SUB // GRP    # 2 bs-groups
DIG = 4                 # d_inner chunks per out_w DMA
NDG = NDI // DIG        # 8 out_w DMA chunks per dm column block

F32 = mybir.dt.float32
BF16 = mybir.dt.bfloat16
SILU = mybir.ActivationFunctionType.Silu


def build_nc():
    nc = bacc.Bacc("TRN2", target_bir_lowering=False, debug=False,
                   num_devices=N_CORES)

    # host-shuffled layouts (see prepare_in_maps):
    #  xt : row t*128+p holds [kt][b]  (b within tile t)     bf16
    #  iw : row di*128+p holds [kt][m]                        bf16
    #  ow : row (dmc*NDG+dg)*128+p holds [s][m]               bf16
    xt_d = nc.dram_tensor("xt", [NBT * P, KT * BT], BF16,
                          kind="ExternalInput").ap()
    iw_d = nc.dram_tensor("iw", [NDI * P, KT * P], BF16,
                          kind="ExternalInput").ap()
    ow_d = nc.dram_tensor("ow", [NDM * NDG * P, DIG * 512], BF16,
                          kind="ExternalInput").ap()
    c_d = nc.dram_tensor("cpb", [P, NDI], F32, kind="ExternalInput").ap()
    b_d = nc.dram_tensor("bpb", [P, NDI], F32, kind="ExternalInput").ap()
    ob_d = nc.dram_tensor("ob", [P, DM], F32, kind="ExternalInput").ap()
    out_d = nc.dram_tensor("out", [BS, DM], F32, kind="ExternalOutput").ap()

    with tile.TileContext(nc) as tc:
        XC = 4                  # kt-chunks per xT tile load
        KC = KT // XC           # k-tiles per chunk
        with (
            tc.tile_pool(name="const", bufs=1) as const,
            tc.tile_pool(name="xT", bufs=2 * XC) as xtp,
            tc.tile_pool(name="g", bufs=1) as gp,
            tc.tile_pool(name="iw", bufs=3) as iwp,
            tc.tile_pool(name="ow", bufs=6) as owp,
            tc.tile_pool(name="osb", bufs=2) as osbp,
            tc.tile_pool(name="psZ", bufs=3, space="PSUM") as psZ,
            tc.tile_pool(name="psO", bufs=5, space="PSUM") as psO,
        ):
            c_sb = const.tile([P, NDI], F32)
            nc.scalar.dma_start(c_sb[:], c_d)
            b_sb = const.tile([P, NDI], F32)
            nc.scalar.dma_start(b_sb[:], b_d)
            ob_sb = const.tile([P, DM], F32)
            nc.scalar.dma_start(ob_sb[:], ob_d)

            g = gp.tile([P, NDI, BT], BF16)

            xTs = {}

            def load_xT_chunk(t, c):
                # 1 MB chunk of k-tiles [KC*c, KC*(c+1)) on the SWDGE ring
                xTs[(t, c)] = xtp.tile([P, KC, BT], BF16, tag="xt", name="xt")
                nc.gpsimd.dma_start(
                    xTs[(t, c)][:],
                    xt_d[t * P:(t + 1) * P,
                         c * KC * BT:(c + 1) * KC * BT].rearrange(
                        "p (kt b) -> p kt b", kt=KC))

            for c in range(XC):
                load_xT_chunk(0, c)
            for t in range(NBT):
                xT = [xTs.pop((t, c)) for c in range(XC)]

                # ---- M1: z^T = in_w^T @ x^T ; g = silu(z*c + b) ----
                # h-split passes: h1's first matmul is 16 slots after
                # act(h0)'s psum buffer frees -> no psZ WAR stalls.
                for di in range(NDI):
                    iw_t = iwp.tile([P, KT, P], BF16, tag="iw", name="iw")
                    nc.sync.dma_start(
                        iw_t[:],
                        iw_d[di * P:(di + 1) * P, :].rearrange(
                            "p (kt m) -> p kt m", kt=KT))
                    for h in range(H):
                        zp = psZ.tile([P, 512], F32, tag="zp", name="zp")
                        for kt in range(KT):
                            nc.tensor.matmul(
                                zp[:],
                                iw_t[:, kt, :],
                                xT[kt // KC][:, kt % KC,
                                             h * 512:(h + 1) * 512],
                                start=(kt == 0), stop=(kt == KT - 1))
                        nc.scalar.activation(
                            g[:, di, h * 512:(h + 1) * 512], zp[:], SILU,
                            bias=b_sb[:, di:di + 1], scale=c_sb[:, di:di + 1])

                # ---- M2: out = g^T @ out_w + out_b ----
                for dmc in range(NDM):
                    if t + 1 < NBT:
                        load_xT_chunk(t + 1, dmc)
                    for grp in range(NGRP):
                        ops = [psO.tile([P, 512], F32, tag="ps_o",
                                        name=f"ops{j}")
                               for j in range(GRP)]
                        for dg in range(NDG):
                            ow_t = owp.tile([P, DIG, 512], BF16, tag="ow",
                                            name="ow")
                            r = (dmc * NDG + dg) * P
                            nc.scalar.dma_start(
                                ow_t[:],
                                ow_d[r:r + P, :].rearrange(
                                    "p (s m) -> p s m", s=DIG))
                            for s in range(DIG):
                                di = dg * DIG + s
                                for j in range(GRP):
                                    bs = grp * GRP + j
                                    nc.tensor.matmul(
                                        ops[j][:],
                                        g[:, di, bs * P:(bs + 1) * P],
                                        ow_t[:, s, :],
                                        start=(di == 0),
                                        stop=(di == NDI - 1))
                        osb = osbp.tile([P, GRP, 512], F32, tag="osb",
                                        name="osb")
                        for j in range(GRP):
                            nc.vector.tensor_tensor(
                                osb[:, j, :], ops[j][:],
                                ob_sb[:, dmc * 512:(dmc + 1) * 512],
                                mybir.AluOpType.add)
                        r0 = t * BT + grp * GRP * P
                        last = (t == NBT - 1 and dmc == NDM - 1
                                and grp == NGRP - 1)
                        eng = nc.sync if last else nc.gpsimd
                        eng.dma_start(
                            out_d[r0:r0 + GRP * P,
                                  dmc * 512:(dmc + 1) * 512].rearrange(
                                      "(s p) n -> p s n", p=P),
                            osb[:])
    nc.compile()
    return nc


_NC_CACHE = {}


def _get_nc():
    if "nc" not in _NC_CACHE:
        _NC_CACHE["nc"] = build_nc()
    return _NC_CACHE["nc"]


def _softplus(v):
    return np.logaddexp(0.0, v)


def prepare_in_maps(inputs):
    """Host-side prep: per-channel collapse, bf16 casts, layout shuffles."""
    import ml_dtypes
    bf16 = ml_dtypes.bfloat16

    x = np.asarray(inputs["x"], np.float32)
    in_w = np.asarray(inputs["in_w"], np.float32)
    out_w = np.asarray(inputs["out_w"], np.float32)

    c = (np.asarray(inputs["conv_w"], np.float32)[:, -1]
         + _softplus(np.asarray(inputs["dt"], np.float32))
         * np.sum(np.asarray(inputs["B"], np.float32)
                  * np.asarray(inputs["C"], np.float32), -1)
         + np.asarray(inputs["Dp"], np.float32))
    b_eff = (np.asarray(inputs["in_b"], np.float32) * c
             + np.asarray(inputs["conv_b"], np.float32))

    c_pb = np.ascontiguousarray(c.reshape(NDI, P).T)
    b_pb = np.ascontiguousarray(b_eff.reshape(NDI, P).T)
    ob_rep = np.ascontiguousarray(np.broadcast_to(
        np.asarray(inputs["out_b"], np.float32), (P, DM)))

    # iw[kt*128+p, di*128+m] -> row di*128+p : [kt][m]
    iw_shuf = np.ascontiguousarray(
        in_w.astype(bf16).reshape(KT, P, NDI, P).transpose(2, 1, 0, 3)
        .reshape(NDI * P, KT * P))
    # ow[dg*512+s*128+p, dmc*512+m] -> row (dmc*NDG+dg)*128+p : [s][m]
    ow_shuf = np.ascontiguousarray(
        out_w.astype(bf16).reshape(NDG, DIG, P, NDM, 512)
        .transpose(3, 0, 2, 1, 4).reshape(NDM * NDG * P, DIG * 512))

    in_maps = []
    for i in range(N_CORES):
        xc = x[i * BS:(i + 1) * BS].astype(bf16)
        # xc[t*BT+b, kt*128+p] -> row t*128+p : [kt][b]
        xt_shuf = np.ascontiguousarray(
            xc.reshape(NBT, BT, KT, P).transpose(0, 3, 2, 1)
            .reshape(NBT * P, KT * BT))
        in_maps.append({
            "xt": xt_shuf,
            "iw": iw_shuf,
            "ow": ow_shuf,
            "cpb": c_pb,
            "bpb": b_pb,
            "ob": ob_rep,
        })
    return in_maps


def kernel(x, in_w, in_b, conv_w, conv_b, A_log, B, C, Dp, dt, out_w, out_b):
    in_maps = prepare_in_maps(dict(
        x=x, in_w=in_w, in_b=in_b, conv_w=conv_w, conv_b=conv_b,
        A_log=A_log, B=B, C=C, Dp=Dp, dt=dt, out_w=out_w, out_b=out_b))
    out = np.empty((B_FULL, DM), dtype=np.float32)
    try:
        nc = _get_nc()
        res = run_bass_kernel_spmd(nc, in_maps, core_ids=list(range(N_CORES)))
        for i in range(N_CORES):
            out[i * BS:(i + 1) * BS] = res.results[i]["out"]
    except Exception:
        # The accelerator occasionally hits a transient unrecoverable fault
        # that poisons this process's PJRT client; a fresh process recovers.
        # Retry the device execution in a subprocess.
        _run_in_subprocess(in_maps, out)
    return out


def _run_in_subprocess(in_maps, out):
    import pickle
    import subprocess
    import sys
    import tempfile

    with tempfile.TemporaryDirectory() as td:
        in_path = f"{td}/in.pkl"
        out_path = f"{td}/out.npy"
        with open(in_path, "wb") as f:
            pickle.dump({"in_maps": in_maps}, f,
                        protocol=pickle.HIGHEST_PROTOCOL)
        for attempt in range(3):
            r = subprocess.run(
                [sys.executable, __file__, "--worker", in_path, out_path],
                capture_output=True)
            if r.returncode == 0:
                break
            if attempt == 2:
                raise RuntimeError(
                    f"device worker failed 3x: {r.stderr[-2000:]!r}")
        out[:] = np.load(out_path)


def _worker_main(in_path, out_path):
    import pickle
    with open(in_path, "rb") as f:
        job = pickle.load(f)
    nc = _get_nc()
    res = run_bass_kernel_spmd(nc, job["in_maps"],
                               core_ids=list(range(N_CORES)))
    out = np.empty((B_FULL, DM), dtype=np.float32)
    for i in range(N_CORES):
        out[i * BS:(i + 1) * BS] = res.results[i]["out"]
    np.save(out_path, out)


if __name__ == "__main__":
    import sys as _sys
    if len(_sys.argv) == 4 and _sys.argv[1] == "--worker":
        _worker_main(_sys.argv[2], _sys.argv[3])
